# revision 38
# baseline (speedup 1.0000x reference)
"""MoE block (top-1 routed 2x conv3x3+BN+ReLU experts) on 8 Trainium2 cores.

Strategy (data-parallel, per sharding hint):
  - Host: gate MLP (16x9 -> 16x6), softmax, top-1 routing, balance loss,
    BN folding into conv weights/bias, per-sample expert-weight gather,
    input zero-padding to 66x66 and layout packing.
  - Device (SPMD, 2 samples/core): each 3x3 conv over [256ch, 64, 64] is
    computed as 18 accumulated matmuls per output tile (2 input-channel
    tiles x 9 taps) using shifted windows of the padded image. ScalarE
    applies bias+ReLU straight out of PSUM.
"""

import os
import sys
import time

import numpy as np

for _p in (
    "/root/.axon_site",
    "/root/.axon_site/_ro/trn_rl_repo",
    "/root/.axon_site/_ro/pypackages",
    "/opt/trn_rl_repo",
):
    if os.path.isdir(_p) and _p not in sys.path:
        sys.path.append(_p)

import ml_dtypes  # noqa: E402

import concourse.bacc as bacc  # noqa: E402
import concourse.mybir as mybir  # noqa: E402
import concourse.tile as tile  # noqa: E402
from concourse.bass_utils import run_bass_kernel_spmd  # noqa: E402

E, C, HID, META, GATE_H = 6, 256, 256, 9, 128
B, H, W = 16, 64, 64
BN_EPS = 1e-5
NCORES = 8
SPC = B // NCORES  # samples per core
HP = H + 2  # zero-padded rows
WP = int(os.environ.get("MOE_WP", "66"))  # padded row stride
CT = C // 128  # input-channel tiles
MT = 2  # output-channel tiles
YB = 8  # output rows per matmul block (N = YB*W = 512)
NYB = H // YB
GRP = 4  # psum banks interleaved per weight load

DT_MODE = os.environ.get("MOE_DT", "fp16")
WARM = int(os.environ.get("MOE_WARM", "16"))    # PE warmup matmul count
WARM_N = int(os.environ.get("MOE_WARM_N", "256"))  # warmup moving width

_NC_CACHE = {}


def _build_nc(dt_mode):
    f32 = mybir.dt.float32
    if dt_mode == "bf16":
        sdt, mm_cast = mybir.dt.bfloat16, None
        big_bufs = 2
    elif dt_mode == "fp16":
        sdt, mm_cast = mybir.dt.float16, None
        big_bufs = 2
    elif dt_mode == "f32":
        sdt, mm_cast = f32, None
        big_bufs = 1
    elif dt_mode == "f32r":
        sdt, mm_cast = mybir.dt.float32r, None
        big_bufs = 1
    else:
        raise ValueError(f"bad MOE_DT {dt_mode}")

    def mm(ap):
        return ap.bitcast(mm_cast) if mm_cast is not None else ap

    nc = bacc.Bacc()
    x_d = nc.declare_dram_parameter("x", [SPC, 128, CT, HP, WP], sdt, isOutput=False)
    w_d = nc.declare_dram_parameter(
        "w", [SPC, 2, MT, 128, CT, 9, 128], sdt, isOutput=False)
    b_d = nc.declare_dram_parameter("bias", [128, SPC * 2 * MT], f32, isOutput=False)
    o_d = nc.declare_dram_parameter("out", [SPC, MT, 128, H, W], f32, isOutput=True)

    with tile.TileContext(nc) as tc:
        with (
            tc.tile_pool(name="io", bufs=2) as io_pool,
            tc.tile_pool(name="biasp", bufs=1) as bias_pool,
            tc.tile_pool(name="psum", bufs=8, space="PSUM") as psum_pool,
        ):
            # DMA bias to a staging tile, then copy on ScalarE: the ACTs'
            # bias dependency becomes same-engine program order (the ACT HW
            # instruction can only encode a single semaphore wait). Issued on
            # gpsimd so the tiny transfer doesn't take an early issue slot in
            # the sync queue feeding the first matmuls.
            bias_dma = bias_pool.tile([128, SPC * 2 * MT], f32, name="bias_dma")
            nc.gpsimd.dma_start(bias_dma[:], b_d[:])
            bias_sb = bias_pool.tile([128, SPC * 2 * MT], f32, name="bias_sb")
            nc.scalar.copy(bias_sb[:], bias_dma[:])
            # Dummy ScalarE op absorbing the same-engine wait on the bias
            # copy, so the first real Activation carries only the PE wait
            # (the ACT HW instruction encodes a single semaphore wait).
            bias_scr = bias_pool.tile([128, 1], f32, name="bias_scr")
            nc.scalar.copy(bias_scr[:], bias_sb[:, 0:1])

            # PE warmup: ~20 matmuls on zeroed SBUF run during the input DMA
            # head, so the HAM clock gate is already at full rate (2.4 GHz)
            # when the first real matmul issues.
            warm_sb = bias_pool.tile([128, 512], sdt, name="warm_sb")
            warm_ap = warm_sb[:]
            if sdt == mybir.dt.float32r:
                nc.vector.memset(warm_ap.bitcast(mybir.dt.float32), 0.0)
            else:
                nc.vector.memset(warm_ap, 0.0)
            warm_ps = psum_pool.tile([128, YB, W], f32, name="warm_ps", tag="ps")
            for _ in range(WARM):
                nc.tensor.matmul(warm_ps[:, 0:WARM_N // W, :],
                                 warm_sb[:, 0:128], warm_sb[:, 0:WARM_N],
                                 start=True, stop=True, skip_group_check=True)

            for s in range(SPC):
                # DMA issue order matters for the pipeline head: the first
                # matmuls need only w1[mo=0] + x rows 0..10.
                w1_t = io_pool.tile([128, CT, 9, 256], sdt, name=f"w1_{s}", tag="w1")
                x_t = io_pool.tile([128, CT, HP, WP], sdt, name=f"x_{s}", tag="x",
                                   bufs=big_bufs)
                nc.sync.dma_start(w1_t[:, 0, :, 0:128], w_d[s, 0, 0, :, 0])
                nc.sync.dma_start(x_t[:, 0, 0:11, :], x_d[s, :, 0, 0:11, :])
                nc.sync.dma_start(x_t[:, 1, 0:11, :], x_d[s, :, 1, 0:11, :])
                nc.sync.dma_start(w1_t[:, 1, :, 0:128], w_d[s, 0, 0, :, 1])
                for r0, r1 in ((11, 35), (35, HP)):
                    for ct in range(CT):
                        nc.sync.dma_start(
                            x_t[:, ct, r0:r1, :], x_d[s, :, ct, r0:r1, :])
                for ct in range(CT):
                    nc.sync.dma_start(w1_t[:, ct, :, 128:256], w_d[s, 0, 1, :, ct])
                w2_t = io_pool.tile([128, CT, 9, 256], sdt, name=f"w2_{s}", tag="w2")
                for mo in range(MT):
                    nc.sync.dma_start(
                        w2_t[:, :, :, mo * 128:(mo + 1) * 128], w_d[s, 1, mo])
                y1_t = io_pool.tile([128, CT, HP, WP], sdt, name=f"y1_{s}", tag="y1",
                                    bufs=big_bufs)
                y2_t = io_pool.tile([128, MT, H, W], f32, name=f"y2_{s}", tag="y2",
                                    bufs=big_bufs)

                # zero the padding ring of the intermediate image
                def ms(ap):
                    if sdt == mybir.dt.float32r:
                        ap = ap.bitcast(mybir.dt.float32)
                    nc.gpsimd.memset(ap, 0.0)

                for ct in range(CT):
                    ms(y1_t[:, ct, 0, :])
                    ms(y1_t[:, ct, HP - 1, :])
                    ms(y1_t[:, ct, 1:HP - 1, 0:1])
                    ms(y1_t[:, ct, 1:HP - 1, 1 + W:2 + W])

                for conv in range(2):
                    src = x_t if conv == 0 else y1_t
                    wt = w1_t if conv == 0 else w2_t
                    for mo in range(MT):
                        bidx = (s * 2 + conv) * MT + mo
                        for yog in range(NYB // GRP):
                            pss = [psum_pool.tile(
                                [128, YB, W], f32,
                                name=f"ps_{s}_{conv}_{mo}_{yog}_{j}", tag="ps")
                                for j in range(GRP)]
                            # interleave GRP psum banks so one weight load
                            # serves GRP consecutive matmuls
                            for k, (ct, off) in enumerate(
                                    (c, o) for c in range(CT) for o in range(9)):
                                ky, kx = divmod(off, 3)
                                lhsT = wt[:, ct, off, mo * 128:(mo + 1) * 128]
                                for j in range(GRP):
                                    yo = yog * GRP + j
                                    rhs = src[:, ct,
                                              yo * YB + ky: yo * YB + ky + YB,
                                              kx: kx + W]
                                    nc.tensor.matmul(
                                        pss[j][:], mm(lhsT), mm(rhs),
                                        start=(k == 0), stop=(k == CT * 9 - 1),
                                        skip_group_check=True)
                            for j in range(GRP):
                                yo = yog * GRP + j
                                if conv == 0:
                                    dst = y1_t[:, mo,
                                               1 + yo * YB: 1 + yo * YB + YB,
                                               1: 1 + W]
                                else:
                                    dst = y2_t[:, mo, yo * YB: (yo + 1) * YB, :]
                                nc.scalar.activation(
                                    dst, pss[j][:],
                                    mybir.ActivationFunctionType.Relu,
                                    bias=bias_sb[:, bidx: bidx + 1])
                                # store finished output rows while later
                                # tiles are still computing
                                if conv == 1:
                                    r0, r1 = yo * YB, yo * YB + YB
                                    nc.sync.dma_start(
                                        o_d[s, mo, :, r0:r1, :],
                                        y2_t[:, mo, r0:r1, :])
    nc.compile()
    return nc


def _get_nc(dt_mode):
    if dt_mode not in _NC_CACHE:
        _NC_CACHE[dt_mode] = _build_nc(dt_mode)
    return _NC_CACHE[dt_mode]


def _host_gate(meta, gate_w1, gate_b1, gate_w2, gate_b2):
    f32 = np.float32
    h = np.maximum(meta.astype(f32) @ gate_w1.astype(f32) + gate_b1.astype(f32), 0.0)
    logits = h @ gate_w2.astype(f32) + gate_b2.astype(f32)
    z = np.exp(logits - logits.max(axis=-1, keepdims=True))
    probs = (z / z.sum(axis=-1, keepdims=True)).astype(f32)
    top1 = np.argmax(probs, axis=-1)
    importance = probs.sum(axis=0) / (probs.sum() + np.float32(1e-8))
    balance = np.std(importance.astype(f32), ddof=1)
    return top1, np.float32(balance)


def _run(inputs, trace=False, dt_mode=None):
    dt_mode = dt_mode or DT_MODE
    f32 = np.float32
    if dt_mode in ("f32", "f32r"):
        np_sdt = f32
    elif dt_mode == "fp16":
        np_sdt = np.float16
    else:
        np_sdt = ml_dtypes.bfloat16

    moe_c4 = np.asarray(inputs["moe_c4"], f32)
    top1, balance = _host_gate(
        np.asarray(inputs["meta"], f32),
        np.asarray(inputs["gate_w1"], f32), np.asarray(inputs["gate_b1"], f32),
        np.asarray(inputs["gate_w2"], f32), np.asarray(inputs["gate_b2"], f32))

    # fold BN into conv weights/bias (per expert)
    c1w = np.asarray(inputs["conv1_w"], f32)
    c2w = np.asarray(inputs["conv2_w"], f32)
    s1 = np.asarray(inputs["bn1_g"], f32) / np.sqrt(np.asarray(inputs["bn1_v"], f32) + BN_EPS)
    s2 = np.asarray(inputs["bn2_g"], f32) / np.sqrt(np.asarray(inputs["bn2_v"], f32) + BN_EPS)
    w1f = c1w * s1[:, :, None, None, None]
    w2f = c2w * s2[:, :, None, None, None]
    b1f = (np.asarray(inputs["conv1_b"], f32) - np.asarray(inputs["bn1_m"], f32)) * s1 \
        + np.asarray(inputs["bn1_b"], f32)
    b2f = (np.asarray(inputs["conv2_b"], f32) - np.asarray(inputs["bn2_m"], f32)) * s2 \
        + np.asarray(inputs["bn2_b"], f32)

    # pack stationary operands: [E, co, ci, ky, kx] -> [E, 128p, ci_t, 9, co]
    def pack_w(wf):
        return np.ascontiguousarray(
            wf.transpose(0, 2, 3, 4, 1)       # [E, ci, ky, kx, co]
            .reshape(E, CT, 128, 9, 256)
            .transpose(0, 2, 1, 3, 4))        # [E, 128, CT, 9, co]

    w1p = pack_w(w1f)
    w2p = pack_w(w2f)

    # padded input, cast to device dtype: [B, C, HP, WP]
    xp = np.zeros((B, C, HP, WP), np_sdt)
    xp[:, :, 1:1 + H, 1:1 + W] = moe_c4.astype(np_sdt)

    def split_mo(wp):
        # [E, 128, CT, 9, 256] -> [E, MT, 128, CT, 9, 128]
        return np.ascontiguousarray(
            wp.reshape(E, 128, CT, 9, MT, 128).transpose(0, 4, 1, 2, 3, 5))

    w_sel1 = split_mo(w1p)[top1].astype(np_sdt)  # [B, MT, 128, CT, 9, 128]
    w_sel2 = split_mo(w2p)[top1].astype(np_sdt)
    b_sel1 = b1f[top1].reshape(B, MT, 128)  # [B, MT, 128]
    b_sel2 = b2f[top1].reshape(B, MT, 128)

    in_maps = []
    for c in range(NCORES):
        sl = slice(c * SPC, (c + 1) * SPC)
        x_core = np.ascontiguousarray(
            xp[sl].reshape(SPC, CT, 128, HP, WP).transpose(0, 2, 1, 3, 4))
        w_core = np.ascontiguousarray(
            np.stack([w_sel1[sl], w_sel2[sl]], axis=1))  # [SPC, 2, MT, 128, CT, 9, 128]
        bias_core = np.ascontiguousarray(
            np.stack([b_sel1[sl], b_sel2[sl]], axis=1)   # [SPC, 2, MT, 128]
            .transpose(3, 0, 1, 2).reshape(128, SPC * 2 * MT)).astype(f32)
        in_maps.append({"x": x_core, "w": w_core, "bias": bias_core})

    nc = _get_nc(dt_mode)
    res = None
    for attempt in range(3):
        try:
            res = run_bass_kernel_spmd(
                nc, in_maps, core_ids=list(range(NCORES)),
                trace=trace, trace_cores=[0] if trace else None)
            break
        except Exception:
            if attempt == 2:
                raise
            time.sleep(5.0)

    out = np.concatenate(
        [r["out"].reshape(SPC, C, H, W) for r in res.results], axis=0)
    return (out.astype(f32), balance), res


def kernel(**inputs):
    (out, balance), _ = _run(inputs)
    return out, balance


# revision 40
# speedup vs baseline: 1.0107x; 1.0107x over previous
"""MoE block (top-1 routed 2x conv3x3+BN+ReLU experts) on 8 Trainium2 cores.

Strategy (data-parallel, per sharding hint):
  - Host: gate MLP (16x9 -> 16x6), softmax, top-1 routing, balance loss,
    BN folding into conv weights/bias, per-sample expert-weight gather,
    input zero-padding to 66x66 and layout packing.
  - Device (SPMD, 2 samples/core): each 3x3 conv over [256ch, 64, 64] is
    computed as 18 accumulated matmuls per output tile (2 input-channel
    tiles x 9 taps) using shifted windows of the padded image. ScalarE
    applies bias+ReLU straight out of PSUM.
"""

import os
import sys
import time

import numpy as np

for _p in (
    "/root/.axon_site",
    "/root/.axon_site/_ro/trn_rl_repo",
    "/root/.axon_site/_ro/pypackages",
    "/opt/trn_rl_repo",
):
    if os.path.isdir(_p) and _p not in sys.path:
        sys.path.append(_p)

import ml_dtypes  # noqa: E402

import concourse.bacc as bacc  # noqa: E402
import concourse.mybir as mybir  # noqa: E402
import concourse.tile as tile  # noqa: E402
from concourse.bass_utils import run_bass_kernel_spmd  # noqa: E402

E, C, HID, META, GATE_H = 6, 256, 256, 9, 128
B, H, W = 16, 64, 64
BN_EPS = 1e-5
NCORES = 8
SPC = B // NCORES  # samples per core
HP = H + 2  # zero-padded rows
WP = int(os.environ.get("MOE_WP", "66"))  # padded row stride
CT = C // 128  # input-channel tiles
MT = 2  # output-channel tiles
YB = 8  # output rows per matmul block (N = YB*W = 512)
NYB = H // YB
GRP = 4  # psum banks interleaved per weight load

DT_MODE = os.environ.get("MOE_DT", "fp16")
WARM = int(os.environ.get("MOE_WARM", "16"))    # PE warmup matmul count
WARM_N = int(os.environ.get("MOE_WARM_N", "256"))  # warmup moving width

_NC_CACHE = {}


def _build_nc(dt_mode):
    f32 = mybir.dt.float32
    if dt_mode == "bf16":
        sdt, mm_cast = mybir.dt.bfloat16, None
        big_bufs = 2
    elif dt_mode == "fp16":
        sdt, mm_cast = mybir.dt.float16, None
        big_bufs = 2
    elif dt_mode == "f32":
        sdt, mm_cast = f32, None
        big_bufs = 1
    elif dt_mode == "f32r":
        sdt, mm_cast = mybir.dt.float32r, None
        big_bufs = 1
    else:
        raise ValueError(f"bad MOE_DT {dt_mode}")

    def mm(ap):
        return ap.bitcast(mm_cast) if mm_cast is not None else ap

    nc = bacc.Bacc()
    x_d = nc.declare_dram_parameter("x", [SPC, 128, CT, HP, WP], sdt, isOutput=False)
    w_d = nc.declare_dram_parameter(
        "w", [SPC, 2, MT, 128, CT, 9, 128], sdt, isOutput=False)
    b_d = nc.declare_dram_parameter("bias", [128, SPC * 2 * MT], f32, isOutput=False)
    o_d = nc.declare_dram_parameter("out", [SPC, MT, 128, H, W], f32, isOutput=True)

    with tile.TileContext(nc) as tc:
        with (
            tc.tile_pool(name="io", bufs=2) as io_pool,
            tc.tile_pool(name="biasp", bufs=1) as bias_pool,
            tc.tile_pool(name="psum", bufs=8, space="PSUM") as psum_pool,
        ):
            # DMA bias to a staging tile, then copy on ScalarE: the ACTs'
            # bias dependency becomes same-engine program order (the ACT HW
            # instruction can only encode a single semaphore wait). Issued on
            # gpsimd so the tiny transfer doesn't take an early issue slot in
            # the sync queue feeding the first matmuls.
            bias_dma = bias_pool.tile([128, SPC * 2 * MT], f32, name="bias_dma")
            nc.gpsimd.dma_start(bias_dma[:], b_d[:])
            bias_sb = bias_pool.tile([128, SPC * 2 * MT], f32, name="bias_sb")
            nc.scalar.copy(bias_sb[:], bias_dma[:])
            # Dummy ScalarE op absorbing the same-engine wait on the bias
            # copy, so the first real Activation carries only the PE wait
            # (the ACT HW instruction encodes a single semaphore wait).
            bias_scr = bias_pool.tile([128, 1], f32, name="bias_scr")
            nc.scalar.copy(bias_scr[:], bias_sb[:, 0:1])

            # PE warmup: ~20 matmuls on zeroed SBUF run during the input DMA
            # head, so the HAM clock gate is already at full rate (2.4 GHz)
            # when the first real matmul issues.
            warm_sb = bias_pool.tile([128, 512], sdt, name="warm_sb")
            warm_ap = warm_sb[:]
            if sdt == mybir.dt.float32r:
                nc.vector.memset(warm_ap.bitcast(mybir.dt.float32), 0.0)
            else:
                nc.vector.memset(warm_ap, 0.0)
            warm_ps = psum_pool.tile([128, YB, W], f32, name="warm_ps", tag="ps")
            for _ in range(WARM):
                nc.tensor.matmul(warm_ps[:, 0:WARM_N // W, :],
                                 warm_sb[:, 0:128], warm_sb[:, 0:WARM_N],
                                 start=True, stop=True, skip_group_check=True)

            for s in range(SPC):
                # DMA issue order matters for the pipeline head: the first
                # matmuls need only w1[mo=0] + x rows 0..10.
                w1_t = io_pool.tile([128, CT, 9, 256], sdt, name=f"w1_{s}", tag="w1")
                x_t = io_pool.tile([128, CT, HP, WP], sdt, name=f"x_{s}", tag="x",
                                   bufs=big_bufs)
                nc.sync.dma_start(w1_t[:, 0, :, 0:128], w_d[s, 0, 0, :, 0])
                nc.sync.dma_start(x_t[:, 0, 0:11, :], x_d[s, :, 0, 0:11, :])
                nc.sync.dma_start(x_t[:, 1, 0:11, :], x_d[s, :, 1, 0:11, :])
                nc.sync.dma_start(w1_t[:, 1, :, 0:128], w_d[s, 0, 0, :, 1])
                for r0, r1 in ((11, 35), (35, HP)):
                    for ct in range(CT):
                        nc.sync.dma_start(
                            x_t[:, ct, r0:r1, :], x_d[s, :, ct, r0:r1, :])
                for ct in range(CT):
                    nc.sync.dma_start(w1_t[:, ct, :, 128:256], w_d[s, 0, 1, :, ct])
                w2_t = io_pool.tile([128, CT, 9, 256], sdt, name=f"w2_{s}", tag="w2")
                for mo in range(MT):
                    nc.sync.dma_start(
                        w2_t[:, :, :, mo * 128:(mo + 1) * 128], w_d[s, 1, mo])
                y1_t = io_pool.tile([128, CT, HP, WP], sdt, name=f"y1_{s}", tag="y1",
                                    bufs=big_bufs)
                y2_t = io_pool.tile([128, MT, H, W], f32, name=f"y2_{s}", tag="y2",
                                    bufs=big_bufs)

                # zero the padding ring of the intermediate image
                def ms(ap):
                    if sdt == mybir.dt.float32r:
                        ap = ap.bitcast(mybir.dt.float32)
                    nc.gpsimd.memset(ap, 0.0)

                for ct in range(CT):
                    ms(y1_t[:, ct, 0, :])
                    ms(y1_t[:, ct, HP - 1, :])
                    ms(y1_t[:, ct, 1:HP - 1, 0:1])
                    ms(y1_t[:, ct, 1:HP - 1, 1 + W:2 + W])

                for conv in range(2):
                    src = x_t if conv == 0 else y1_t
                    wt = w1_t if conv == 0 else w2_t
                    for mo in range(MT):
                        bidx = (s * 2 + conv) * MT + mo
                        for yog in range(NYB // GRP):
                            pss = [psum_pool.tile(
                                [128, YB, W], f32,
                                name=f"ps_{s}_{conv}_{mo}_{yog}_{j}", tag="ps")
                                for j in range(GRP)]
                            # interleave GRP psum banks so one weight load
                            # serves GRP consecutive matmuls
                            for k, (ct, off) in enumerate(
                                    (c, o) for c in range(CT) for o in range(9)):
                                ky, kx = divmod(off, 3)
                                lhsT = wt[:, ct, off, mo * 128:(mo + 1) * 128]
                                for j in range(GRP):
                                    yo = yog * GRP + j
                                    rhs = src[:, ct,
                                              yo * YB + ky: yo * YB + ky + YB,
                                              kx: kx + W]
                                    nc.tensor.matmul(
                                        pss[j][:], mm(lhsT), mm(rhs),
                                        start=(k == 0), stop=(k == CT * 9 - 1),
                                        skip_group_check=True)
                            for j in range(GRP):
                                yo = yog * GRP + j
                                if conv == 0:
                                    dst = y1_t[:, mo,
                                               1 + yo * YB: 1 + yo * YB + YB,
                                               1: 1 + W]
                                else:
                                    dst = y2_t[:, mo, yo * YB: (yo + 1) * YB, :]
                                nc.scalar.activation(
                                    dst, pss[j][:],
                                    mybir.ActivationFunctionType.Relu,
                                    bias=bias_sb[:, bidx: bidx + 1])
                                # store finished output rows while later
                                # tiles are still computing
                                if conv == 1:
                                    r0, r1 = yo * YB, yo * YB + YB
                                    nc.sync.dma_start(
                                        o_d[s, mo, :, r0:r1, :],
                                        y2_t[:, mo, r0:r1, :])
    nc.compile()
    return nc


def _get_nc(dt_mode):
    if dt_mode not in _NC_CACHE:
        _NC_CACHE[dt_mode] = _build_nc(dt_mode)
    return _NC_CACHE[dt_mode]


def _host_gate(meta, gate_w1, gate_b1, gate_w2, gate_b2):
    f32 = np.float32
    h = np.maximum(meta.astype(f32) @ gate_w1.astype(f32) + gate_b1.astype(f32), 0.0)
    logits = h @ gate_w2.astype(f32) + gate_b2.astype(f32)
    z = np.exp(logits - logits.max(axis=-1, keepdims=True))
    probs = (z / z.sum(axis=-1, keepdims=True)).astype(f32)
    top1 = np.argmax(probs, axis=-1)
    importance = probs.sum(axis=0) / (probs.sum() + np.float32(1e-8))
    balance = np.std(importance.astype(f32), ddof=1)
    return top1, np.float32(balance)


def _run(inputs, trace=False, dt_mode=None, trace_cores=None):
    dt_mode = dt_mode or DT_MODE
    f32 = np.float32
    if dt_mode in ("f32", "f32r"):
        np_sdt = f32
    elif dt_mode == "fp16":
        np_sdt = np.float16
    else:
        np_sdt = ml_dtypes.bfloat16

    moe_c4 = np.asarray(inputs["moe_c4"], f32)
    top1, balance = _host_gate(
        np.asarray(inputs["meta"], f32),
        np.asarray(inputs["gate_w1"], f32), np.asarray(inputs["gate_b1"], f32),
        np.asarray(inputs["gate_w2"], f32), np.asarray(inputs["gate_b2"], f32))

    # fold BN into conv weights/bias (per expert)
    c1w = np.asarray(inputs["conv1_w"], f32)
    c2w = np.asarray(inputs["conv2_w"], f32)
    s1 = np.asarray(inputs["bn1_g"], f32) / np.sqrt(np.asarray(inputs["bn1_v"], f32) + BN_EPS)
    s2 = np.asarray(inputs["bn2_g"], f32) / np.sqrt(np.asarray(inputs["bn2_v"], f32) + BN_EPS)
    w1f = c1w * s1[:, :, None, None, None]
    w2f = c2w * s2[:, :, None, None, None]
    b1f = (np.asarray(inputs["conv1_b"], f32) - np.asarray(inputs["bn1_m"], f32)) * s1 \
        + np.asarray(inputs["bn1_b"], f32)
    b2f = (np.asarray(inputs["conv2_b"], f32) - np.asarray(inputs["bn2_m"], f32)) * s2 \
        + np.asarray(inputs["bn2_b"], f32)

    # pack stationary operands: [E, co, ci, ky, kx] -> [E, 128p, ci_t, 9, co]
    def pack_w(wf):
        return np.ascontiguousarray(
            wf.transpose(0, 2, 3, 4, 1)       # [E, ci, ky, kx, co]
            .reshape(E, CT, 128, 9, 256)
            .transpose(0, 2, 1, 3, 4))        # [E, 128, CT, 9, co]

    w1p = pack_w(w1f)
    w2p = pack_w(w2f)

    # padded input, cast to device dtype: [B, C, HP, WP]
    xp = np.zeros((B, C, HP, WP), np_sdt)
    xp[:, :, 1:1 + H, 1:1 + W] = moe_c4.astype(np_sdt)

    def split_mo(wp):
        # [E, 128, CT, 9, 256] -> [E, MT, 128, CT, 9, 128]
        return np.ascontiguousarray(
            wp.reshape(E, 128, CT, 9, MT, 128).transpose(0, 4, 1, 2, 3, 5))

    w_sel1 = split_mo(w1p)[top1].astype(np_sdt)  # [B, MT, 128, CT, 9, 128]
    w_sel2 = split_mo(w2p)[top1].astype(np_sdt)
    b_sel1 = b1f[top1].reshape(B, MT, 128)  # [B, MT, 128]
    b_sel2 = b2f[top1].reshape(B, MT, 128)

    in_maps = []
    for c in range(NCORES):
        sl = slice(c * SPC, (c + 1) * SPC)
        x_core = np.ascontiguousarray(
            xp[sl].reshape(SPC, CT, 128, HP, WP).transpose(0, 2, 1, 3, 4))
        w_core = np.ascontiguousarray(
            np.stack([w_sel1[sl], w_sel2[sl]], axis=1))  # [SPC, 2, MT, 128, CT, 9, 128]
        bias_core = np.ascontiguousarray(
            np.stack([b_sel1[sl], b_sel2[sl]], axis=1)   # [SPC, 2, MT, 128]
            .transpose(3, 0, 1, 2).reshape(128, SPC * 2 * MT)).astype(f32)
        in_maps.append({"x": x_core, "w": w_core, "bias": bias_core})

    nc = _get_nc(dt_mode)
    res = None
    for attempt in range(3):
        try:
            res = run_bass_kernel_spmd(
                nc, in_maps, core_ids=list(range(NCORES)),
                trace=trace,
                trace_cores=(trace_cores or [0]) if trace else None)
            break
        except Exception:
            if attempt == 2:
                raise
            time.sleep(5.0)

    out = np.concatenate(
        [r["out"].reshape(SPC, C, H, W) for r in res.results], axis=0)
    return (out.astype(f32), balance), res


def kernel(**inputs):
    (out, balance), _ = _run(inputs)
    return out, balance


# revision 41
# speedup vs baseline: 1.0157x; 1.0049x over previous
"""MoE block (top-1 routed 2x conv3x3+BN+ReLU experts) on 8 Trainium2 cores.

Strategy (data-parallel, per sharding hint):
  - Host: gate MLP (16x9 -> 16x6), softmax, top-1 routing, balance loss,
    BN folding into conv weights/bias, per-sample expert-weight gather,
    input zero-padding to 66x66 and layout packing.
  - Device (SPMD, 2 samples/core): each 3x3 conv over [256ch, 64, 64] is
    computed as 18 accumulated matmuls per output tile (2 input-channel
    tiles x 9 taps) using shifted windows of the padded image. ScalarE
    applies bias+ReLU straight out of PSUM.
"""

import os
import sys
import time

import numpy as np

for _p in (
    "/root/.axon_site",
    "/root/.axon_site/_ro/trn_rl_repo",
    "/root/.axon_site/_ro/pypackages",
    "/opt/trn_rl_repo",
):
    if os.path.isdir(_p) and _p not in sys.path:
        sys.path.append(_p)

import ml_dtypes  # noqa: E402

import concourse.bacc as bacc  # noqa: E402
import concourse.mybir as mybir  # noqa: E402
import concourse.tile as tile  # noqa: E402
from concourse.bass_utils import run_bass_kernel_spmd  # noqa: E402

E, C, HID, META, GATE_H = 6, 256, 256, 9, 128
B, H, W = 16, 64, 64
BN_EPS = 1e-5
NCORES = 8
SPC = B // NCORES  # samples per core
HP = H + 2  # zero-padded rows
WP = int(os.environ.get("MOE_WP", "66"))  # padded row stride
CT = C // 128  # input-channel tiles
MT = 2  # output-channel tiles
YB = 8  # output rows per matmul block (N = YB*W = 512)
NYB = H // YB
GRP = int(os.environ.get("MOE_GRP", "1"))  # psum banks interleaved per group

DT_MODE = os.environ.get("MOE_DT", "fp16")
WARM = int(os.environ.get("MOE_WARM", "16"))    # PE warmup matmul count
WARM_N = int(os.environ.get("MOE_WARM_N", "256"))  # warmup moving width

_NC_CACHE = {}


def _build_nc(dt_mode):
    f32 = mybir.dt.float32
    if dt_mode == "bf16":
        sdt, mm_cast = mybir.dt.bfloat16, None
        big_bufs = 2
    elif dt_mode == "fp16":
        sdt, mm_cast = mybir.dt.float16, None
        big_bufs = 2
    elif dt_mode == "f32":
        sdt, mm_cast = f32, None
        big_bufs = 1
    elif dt_mode == "f32r":
        sdt, mm_cast = mybir.dt.float32r, None
        big_bufs = 1
    else:
        raise ValueError(f"bad MOE_DT {dt_mode}")

    def mm(ap):
        return ap.bitcast(mm_cast) if mm_cast is not None else ap

    nc = bacc.Bacc()
    x_d = nc.declare_dram_parameter("x", [SPC, 128, CT, HP, WP], sdt, isOutput=False)
    w_d = nc.declare_dram_parameter(
        "w", [SPC, 2, MT, 128, CT, 9, 128], sdt, isOutput=False)
    b_d = nc.declare_dram_parameter("bias", [128, SPC * 2 * MT], f32, isOutput=False)
    o_d = nc.declare_dram_parameter("out", [SPC, MT, 128, H, W], f32, isOutput=True)

    with tile.TileContext(nc) as tc:
        with (
            tc.tile_pool(name="io", bufs=2) as io_pool,
            tc.tile_pool(name="biasp", bufs=1) as bias_pool,
            tc.tile_pool(name="psum", bufs=8, space="PSUM") as psum_pool,
        ):
            # DMA bias to a staging tile, then copy on ScalarE: the ACTs'
            # bias dependency becomes same-engine program order (the ACT HW
            # instruction can only encode a single semaphore wait). Issued on
            # gpsimd so the tiny transfer doesn't take an early issue slot in
            # the sync queue feeding the first matmuls.
            bias_dma = bias_pool.tile([128, SPC * 2 * MT], f32, name="bias_dma")
            nc.gpsimd.dma_start(bias_dma[:], b_d[:])
            bias_sb = bias_pool.tile([128, SPC * 2 * MT], f32, name="bias_sb")
            nc.scalar.copy(bias_sb[:], bias_dma[:])
            # Dummy ScalarE op absorbing the same-engine wait on the bias
            # copy, so the first real Activation carries only the PE wait
            # (the ACT HW instruction encodes a single semaphore wait).
            bias_scr = bias_pool.tile([128, 1], f32, name="bias_scr")
            nc.scalar.copy(bias_scr[:], bias_sb[:, 0:1])

            # PE warmup: ~20 matmuls on zeroed SBUF run during the input DMA
            # head, so the HAM clock gate is already at full rate (2.4 GHz)
            # when the first real matmul issues.
            warm_sb = bias_pool.tile([128, 512], sdt, name="warm_sb")
            warm_ap = warm_sb[:]
            if sdt == mybir.dt.float32r:
                nc.vector.memset(warm_ap.bitcast(mybir.dt.float32), 0.0)
            else:
                nc.vector.memset(warm_ap, 0.0)
            warm_ps = psum_pool.tile([128, YB, W], f32, name="warm_ps", tag="ps")
            for _ in range(WARM):
                nc.tensor.matmul(warm_ps[:, 0:WARM_N // W, :],
                                 warm_sb[:, 0:128], warm_sb[:, 0:WARM_N],
                                 start=True, stop=True, skip_group_check=True)

            for s in range(SPC):
                # DMA issue order matters for the pipeline head: the first
                # matmuls need only w1[mo=0] + x rows 0..10.
                w1_t = io_pool.tile([128, CT, 9, 256], sdt, name=f"w1_{s}", tag="w1")
                x_t = io_pool.tile([128, CT, HP, WP], sdt, name=f"x_{s}", tag="x",
                                   bufs=big_bufs)
                nc.sync.dma_start(w1_t[:, 0, :, 0:128], w_d[s, 0, 0, :, 0])
                nc.sync.dma_start(x_t[:, 0, 0:11, :], x_d[s, :, 0, 0:11, :])
                nc.sync.dma_start(x_t[:, 1, 0:11, :], x_d[s, :, 1, 0:11, :])
                nc.sync.dma_start(w1_t[:, 1, :, 0:128], w_d[s, 0, 0, :, 1])
                for r0, r1 in ((11, 35), (35, HP)):
                    for ct in range(CT):
                        nc.sync.dma_start(
                            x_t[:, ct, r0:r1, :], x_d[s, :, ct, r0:r1, :])
                for ct in range(CT):
                    nc.sync.dma_start(w1_t[:, ct, :, 128:256], w_d[s, 0, 1, :, ct])
                w2_t = io_pool.tile([128, CT, 9, 256], sdt, name=f"w2_{s}", tag="w2")
                for mo in range(MT):
                    nc.sync.dma_start(
                        w2_t[:, :, :, mo * 128:(mo + 1) * 128], w_d[s, 1, mo])
                y1_t = io_pool.tile([128, CT, HP, WP], sdt, name=f"y1_{s}", tag="y1",
                                    bufs=big_bufs)
                y2_t = io_pool.tile([128, MT, H, W], f32, name=f"y2_{s}", tag="y2",
                                    bufs=big_bufs)

                # zero the padding ring of the intermediate image
                def ms(ap):
                    if sdt == mybir.dt.float32r:
                        ap = ap.bitcast(mybir.dt.float32)
                    nc.gpsimd.memset(ap, 0.0)

                for ct in range(CT):
                    ms(y1_t[:, ct, 0, :])
                    ms(y1_t[:, ct, HP - 1, :])
                    ms(y1_t[:, ct, 1:HP - 1, 0:1])
                    ms(y1_t[:, ct, 1:HP - 1, 1 + W:2 + W])

                for conv in range(2):
                    src = x_t if conv == 0 else y1_t
                    wt = w1_t if conv == 0 else w2_t
                    for mo in range(MT):
                        bidx = (s * 2 + conv) * MT + mo
                        for yog in range(NYB // GRP):
                            pss = [psum_pool.tile(
                                [128, YB, W], f32,
                                name=f"ps_{s}_{conv}_{mo}_{yog}_{j}", tag="ps")
                                for j in range(GRP)]
                            # interleave GRP psum banks so one weight load
                            # serves GRP consecutive matmuls
                            for k, (ct, off) in enumerate(
                                    (c, o) for c in range(CT) for o in range(9)):
                                ky, kx = divmod(off, 3)
                                lhsT = wt[:, ct, off, mo * 128:(mo + 1) * 128]
                                for j in range(GRP):
                                    yo = yog * GRP + j
                                    rhs = src[:, ct,
                                              yo * YB + ky: yo * YB + ky + YB,
                                              kx: kx + W]
                                    nc.tensor.matmul(
                                        pss[j][:], mm(lhsT), mm(rhs),
                                        start=(k == 0), stop=(k == CT * 9 - 1),
                                        skip_group_check=True)
                            for j in range(GRP):
                                yo = yog * GRP + j
                                if conv == 0:
                                    dst = y1_t[:, mo,
                                               1 + yo * YB: 1 + yo * YB + YB,
                                               1: 1 + W]
                                else:
                                    dst = y2_t[:, mo, yo * YB: (yo + 1) * YB, :]
                                nc.scalar.activation(
                                    dst, pss[j][:],
                                    mybir.ActivationFunctionType.Relu,
                                    bias=bias_sb[:, bidx: bidx + 1])
                                # store finished output rows while later
                                # tiles are still computing
                                if conv == 1:
                                    r0, r1 = yo * YB, yo * YB + YB
                                    nc.sync.dma_start(
                                        o_d[s, mo, :, r0:r1, :],
                                        y2_t[:, mo, r0:r1, :])
    nc.compile()
    return nc


def _get_nc(dt_mode):
    if dt_mode not in _NC_CACHE:
        _NC_CACHE[dt_mode] = _build_nc(dt_mode)
    return _NC_CACHE[dt_mode]


def _host_gate(meta, gate_w1, gate_b1, gate_w2, gate_b2):
    f32 = np.float32
    h = np.maximum(meta.astype(f32) @ gate_w1.astype(f32) + gate_b1.astype(f32), 0.0)
    logits = h @ gate_w2.astype(f32) + gate_b2.astype(f32)
    z = np.exp(logits - logits.max(axis=-1, keepdims=True))
    probs = (z / z.sum(axis=-1, keepdims=True)).astype(f32)
    top1 = np.argmax(probs, axis=-1)
    importance = probs.sum(axis=0) / (probs.sum() + np.float32(1e-8))
    balance = np.std(importance.astype(f32), ddof=1)
    return top1, np.float32(balance)


def _run(inputs, trace=False, dt_mode=None, trace_cores=None):
    dt_mode = dt_mode or DT_MODE
    f32 = np.float32
    if dt_mode in ("f32", "f32r"):
        np_sdt = f32
    elif dt_mode == "fp16":
        np_sdt = np.float16
    else:
        np_sdt = ml_dtypes.bfloat16

    moe_c4 = np.asarray(inputs["moe_c4"], f32)
    top1, balance = _host_gate(
        np.asarray(inputs["meta"], f32),
        np.asarray(inputs["gate_w1"], f32), np.asarray(inputs["gate_b1"], f32),
        np.asarray(inputs["gate_w2"], f32), np.asarray(inputs["gate_b2"], f32))

    # fold BN into conv weights/bias (per expert)
    c1w = np.asarray(inputs["conv1_w"], f32)
    c2w = np.asarray(inputs["conv2_w"], f32)
    s1 = np.asarray(inputs["bn1_g"], f32) / np.sqrt(np.asarray(inputs["bn1_v"], f32) + BN_EPS)
    s2 = np.asarray(inputs["bn2_g"], f32) / np.sqrt(np.asarray(inputs["bn2_v"], f32) + BN_EPS)
    w1f = c1w * s1[:, :, None, None, None]
    w2f = c2w * s2[:, :, None, None, None]
    b1f = (np.asarray(inputs["conv1_b"], f32) - np.asarray(inputs["bn1_m"], f32)) * s1 \
        + np.asarray(inputs["bn1_b"], f32)
    b2f = (np.asarray(inputs["conv2_b"], f32) - np.asarray(inputs["bn2_m"], f32)) * s2 \
        + np.asarray(inputs["bn2_b"], f32)

    # pack stationary operands: [E, co, ci, ky, kx] -> [E, 128p, ci_t, 9, co]
    def pack_w(wf):
        return np.ascontiguousarray(
            wf.transpose(0, 2, 3, 4, 1)       # [E, ci, ky, kx, co]
            .reshape(E, CT, 128, 9, 256)
            .transpose(0, 2, 1, 3, 4))        # [E, 128, CT, 9, co]

    w1p = pack_w(w1f)
    w2p = pack_w(w2f)

    # padded input, cast to device dtype: [B, C, HP, WP]
    xp = np.zeros((B, C, HP, WP), np_sdt)
    xp[:, :, 1:1 + H, 1:1 + W] = moe_c4.astype(np_sdt)

    def split_mo(wp):
        # [E, 128, CT, 9, 256] -> [E, MT, 128, CT, 9, 128]
        return np.ascontiguousarray(
            wp.reshape(E, 128, CT, 9, MT, 128).transpose(0, 4, 1, 2, 3, 5))

    w_sel1 = split_mo(w1p)[top1].astype(np_sdt)  # [B, MT, 128, CT, 9, 128]
    w_sel2 = split_mo(w2p)[top1].astype(np_sdt)
    b_sel1 = b1f[top1].reshape(B, MT, 128)  # [B, MT, 128]
    b_sel2 = b2f[top1].reshape(B, MT, 128)

    in_maps = []
    for c in range(NCORES):
        sl = slice(c * SPC, (c + 1) * SPC)
        x_core = np.ascontiguousarray(
            xp[sl].reshape(SPC, CT, 128, HP, WP).transpose(0, 2, 1, 3, 4))
        w_core = np.ascontiguousarray(
            np.stack([w_sel1[sl], w_sel2[sl]], axis=1))  # [SPC, 2, MT, 128, CT, 9, 128]
        bias_core = np.ascontiguousarray(
            np.stack([b_sel1[sl], b_sel2[sl]], axis=1)   # [SPC, 2, MT, 128]
            .transpose(3, 0, 1, 2).reshape(128, SPC * 2 * MT)).astype(f32)
        in_maps.append({"x": x_core, "w": w_core, "bias": bias_core})

    nc = _get_nc(dt_mode)
    res = None
    for attempt in range(3):
        try:
            res = run_bass_kernel_spmd(
                nc, in_maps, core_ids=list(range(NCORES)),
                trace=trace,
                trace_cores=(trace_cores or [0]) if trace else None)
            break
        except Exception:
            if attempt == 2:
                raise
            time.sleep(5.0)

    out = np.concatenate(
        [r["out"].reshape(SPC, C, H, W) for r in res.results], axis=0)
    return (out.astype(f32), balance), res


def kernel(**inputs):
    (out, balance), _ = _run(inputs)
    return out, balance


# revision 43
# speedup vs baseline: 1.0174x; 1.0017x over previous
"""MoE block (top-1 routed 2x conv3x3+BN+ReLU experts) on 8 Trainium2 cores.

Strategy (data-parallel, per sharding hint):
  - Host: gate MLP (16x9 -> 16x6), softmax, top-1 routing, balance loss,
    BN folding into conv weights/bias, per-sample expert-weight gather,
    input zero-padding to 66x66 and layout packing.
  - Device (SPMD, 2 samples/core): each 3x3 conv over [256ch, 64, 64] is
    computed as 18 accumulated matmuls per output tile (2 input-channel
    tiles x 9 taps) using shifted windows of the padded image. ScalarE
    applies bias+ReLU straight out of PSUM.
"""

import os
import sys
import time

import numpy as np

for _p in (
    "/root/.axon_site",
    "/root/.axon_site/_ro/trn_rl_repo",
    "/root/.axon_site/_ro/pypackages",
    "/opt/trn_rl_repo",
):
    if os.path.isdir(_p) and _p not in sys.path:
        sys.path.append(_p)

import ml_dtypes  # noqa: E402

import concourse.bacc as bacc  # noqa: E402
import concourse.mybir as mybir  # noqa: E402
import concourse.tile as tile  # noqa: E402
from concourse.bass_utils import run_bass_kernel_spmd  # noqa: E402

E, C, HID, META, GATE_H = 6, 256, 256, 9, 128
B, H, W = 16, 64, 64
BN_EPS = 1e-5
NCORES = 8
SPC = B // NCORES  # samples per core
HP = H + 2  # zero-padded rows
WP = int(os.environ.get("MOE_WP", "66"))  # padded row stride
CT = C // 128  # input-channel tiles
MT = 2  # output-channel tiles
YB = 8  # output rows per matmul block (N = YB*W = 512)
NYB = H // YB
GRP = int(os.environ.get("MOE_GRP", "2"))  # psum banks interleaved per group

DT_MODE = os.environ.get("MOE_DT", "fp16")
WARM = int(os.environ.get("MOE_WARM", "16"))    # PE warmup matmul count
WARM_N = int(os.environ.get("MOE_WARM_N", "256"))  # warmup moving width

_NC_CACHE = {}


def _build_nc(dt_mode):
    f32 = mybir.dt.float32
    if dt_mode == "bf16":
        sdt, mm_cast = mybir.dt.bfloat16, None
        big_bufs = 2
    elif dt_mode == "fp16":
        sdt, mm_cast = mybir.dt.float16, None
        big_bufs = 2
    elif dt_mode == "f32":
        sdt, mm_cast = f32, None
        big_bufs = 1
    elif dt_mode == "f32r":
        sdt, mm_cast = mybir.dt.float32r, None
        big_bufs = 1
    else:
        raise ValueError(f"bad MOE_DT {dt_mode}")

    def mm(ap):
        return ap.bitcast(mm_cast) if mm_cast is not None else ap

    nc = bacc.Bacc()
    x_d = nc.declare_dram_parameter("x", [SPC, 128, CT, HP, WP], sdt, isOutput=False)
    w_d = nc.declare_dram_parameter(
        "w", [SPC, 2, MT, 128, CT, 9, 128], sdt, isOutput=False)
    b_d = nc.declare_dram_parameter("bias", [128, SPC * 2 * MT], f32, isOutput=False)
    o_d = nc.declare_dram_parameter("out", [SPC, MT, 128, H, W], f32, isOutput=True)

    with tile.TileContext(nc) as tc:
        with (
            tc.tile_pool(name="io", bufs=2) as io_pool,
            tc.tile_pool(name="biasp", bufs=1) as bias_pool,
            tc.tile_pool(name="psum", bufs=8, space="PSUM") as psum_pool,
        ):
            # DMA bias to a staging tile, then copy on ScalarE: the ACTs'
            # bias dependency becomes same-engine program order (the ACT HW
            # instruction can only encode a single semaphore wait). Issued on
            # gpsimd so the tiny transfer doesn't take an early issue slot in
            # the sync queue feeding the first matmuls.
            bias_dma = bias_pool.tile([128, SPC * 2 * MT], f32, name="bias_dma")
            nc.gpsimd.dma_start(bias_dma[:], b_d[:])
            bias_sb = bias_pool.tile([128, SPC * 2 * MT], f32, name="bias_sb")
            nc.scalar.copy(bias_sb[:], bias_dma[:])
            # Dummy ScalarE op absorbing the same-engine wait on the bias
            # copy, so the first real Activation carries only the PE wait
            # (the ACT HW instruction encodes a single semaphore wait).
            bias_scr = bias_pool.tile([128, 1], f32, name="bias_scr")
            nc.scalar.copy(bias_scr[:], bias_sb[:, 0:1])

            # PE warmup: ~20 matmuls on zeroed SBUF run during the input DMA
            # head, so the HAM clock gate is already at full rate (2.4 GHz)
            # when the first real matmul issues.
            warm_sb = bias_pool.tile([128, 512], sdt, name="warm_sb")
            warm_ap = warm_sb[:]
            if sdt == mybir.dt.float32r:
                nc.vector.memset(warm_ap.bitcast(mybir.dt.float32), 0.0)
            else:
                nc.vector.memset(warm_ap, 0.0)
            warm_ps = psum_pool.tile([128, YB, W], f32, name="warm_ps", tag="ps")
            for _ in range(WARM):
                nc.tensor.matmul(warm_ps[:, 0:WARM_N // W, :],
                                 warm_sb[:, 0:128], warm_sb[:, 0:WARM_N],
                                 start=True, stop=True, skip_group_check=True)

            for s in range(SPC):
                # DMA issue order matters for the pipeline head: the first
                # matmuls need only w1[mo=0] + x rows 0..10.
                w1_t = io_pool.tile([128, CT, 9, 256], sdt, name=f"w1_{s}", tag="w1")
                x_t = io_pool.tile([128, CT, HP, WP], sdt, name=f"x_{s}", tag="x",
                                   bufs=big_bufs)
                nc.sync.dma_start(w1_t[:, 0, :, 0:128], w_d[s, 0, 0, :, 0])
                nc.sync.dma_start(x_t[:, 0, 0:19, :], x_d[s, :, 0, 0:19, :])
                nc.sync.dma_start(x_t[:, 1, 0:19, :], x_d[s, :, 1, 0:19, :])
                nc.sync.dma_start(w1_t[:, 1, :, 0:128], w_d[s, 0, 0, :, 1])
                for r0, r1 in ((19, 35), (35, HP)):
                    for ct in range(CT):
                        nc.sync.dma_start(
                            x_t[:, ct, r0:r1, :], x_d[s, :, ct, r0:r1, :])
                for ct in range(CT):
                    nc.sync.dma_start(w1_t[:, ct, :, 128:256], w_d[s, 0, 1, :, ct])
                w2_t = io_pool.tile([128, CT, 9, 256], sdt, name=f"w2_{s}", tag="w2")
                for mo in range(MT):
                    nc.sync.dma_start(
                        w2_t[:, :, :, mo * 128:(mo + 1) * 128], w_d[s, 1, mo])
                y1_t = io_pool.tile([128, CT, HP, WP], sdt, name=f"y1_{s}", tag="y1",
                                    bufs=big_bufs)
                y2_t = io_pool.tile([128, MT, H, W], f32, name=f"y2_{s}", tag="y2",
                                    bufs=big_bufs)

                # zero the padding ring of the intermediate image
                def ms(ap):
                    if sdt == mybir.dt.float32r:
                        ap = ap.bitcast(mybir.dt.float32)
                    nc.gpsimd.memset(ap, 0.0)

                for ct in range(CT):
                    ms(y1_t[:, ct, 0, :])
                    ms(y1_t[:, ct, HP - 1, :])
                    ms(y1_t[:, ct, 1:HP - 1, 0:1])
                    ms(y1_t[:, ct, 1:HP - 1, 1 + W:2 + W])

                for conv in range(2):
                    src = x_t if conv == 0 else y1_t
                    wt = w1_t if conv == 0 else w2_t
                    for mo in range(MT):
                        bidx = (s * 2 + conv) * MT + mo
                        for yog in range(NYB // GRP):
                            pss = [psum_pool.tile(
                                [128, YB, W], f32,
                                name=f"ps_{s}_{conv}_{mo}_{yog}_{j}", tag="ps")
                                for j in range(GRP)]
                            # ct outer / banks middle / taps inner: all ct=0
                            # matmuls of the group run before any ct=1 data
                            # is needed (hides the ct1 DMA at the kernel
                            # head), and the psum bank only switches every 9
                            # matmuls.
                            for ct in range(CT):
                                for j in range(GRP):
                                    yo = yog * GRP + j
                                    for off in range(9):
                                        ky, kx = divmod(off, 3)
                                        lhsT = wt[:, ct, off,
                                                  mo * 128:(mo + 1) * 128]
                                        rhs = src[:, ct,
                                                  yo * YB + ky: yo * YB + ky + YB,
                                                  kx: kx + W]
                                        nc.tensor.matmul(
                                            pss[j][:], mm(lhsT), mm(rhs),
                                            start=(ct == 0 and off == 0),
                                            stop=(ct == CT - 1 and off == 8),
                                            skip_group_check=True)
                            for j in range(GRP):
                                yo = yog * GRP + j
                                if conv == 0:
                                    dst = y1_t[:, mo,
                                               1 + yo * YB: 1 + yo * YB + YB,
                                               1: 1 + W]
                                else:
                                    dst = y2_t[:, mo, yo * YB: (yo + 1) * YB, :]
                                nc.scalar.activation(
                                    dst, pss[j][:],
                                    mybir.ActivationFunctionType.Relu,
                                    bias=bias_sb[:, bidx: bidx + 1])
                                # store finished output rows while later
                                # tiles are still computing
                                if conv == 1:
                                    r0, r1 = yo * YB, yo * YB + YB
                                    nc.sync.dma_start(
                                        o_d[s, mo, :, r0:r1, :],
                                        y2_t[:, mo, r0:r1, :])
    nc.compile()
    return nc


def _get_nc(dt_mode):
    if dt_mode not in _NC_CACHE:
        _NC_CACHE[dt_mode] = _build_nc(dt_mode)
    return _NC_CACHE[dt_mode]


def _host_gate(meta, gate_w1, gate_b1, gate_w2, gate_b2):
    f32 = np.float32
    h = np.maximum(meta.astype(f32) @ gate_w1.astype(f32) + gate_b1.astype(f32), 0.0)
    logits = h @ gate_w2.astype(f32) + gate_b2.astype(f32)
    z = np.exp(logits - logits.max(axis=-1, keepdims=True))
    probs = (z / z.sum(axis=-1, keepdims=True)).astype(f32)
    top1 = np.argmax(probs, axis=-1)
    importance = probs.sum(axis=0) / (probs.sum() + np.float32(1e-8))
    balance = np.std(importance.astype(f32), ddof=1)
    return top1, np.float32(balance)


def _run(inputs, trace=False, dt_mode=None, trace_cores=None):
    dt_mode = dt_mode or DT_MODE
    f32 = np.float32
    if dt_mode in ("f32", "f32r"):
        np_sdt = f32
    elif dt_mode == "fp16":
        np_sdt = np.float16
    else:
        np_sdt = ml_dtypes.bfloat16

    moe_c4 = np.asarray(inputs["moe_c4"], f32)
    top1, balance = _host_gate(
        np.asarray(inputs["meta"], f32),
        np.asarray(inputs["gate_w1"], f32), np.asarray(inputs["gate_b1"], f32),
        np.asarray(inputs["gate_w2"], f32), np.asarray(inputs["gate_b2"], f32))

    # fold BN into conv weights/bias (per expert)
    c1w = np.asarray(inputs["conv1_w"], f32)
    c2w = np.asarray(inputs["conv2_w"], f32)
    s1 = np.asarray(inputs["bn1_g"], f32) / np.sqrt(np.asarray(inputs["bn1_v"], f32) + BN_EPS)
    s2 = np.asarray(inputs["bn2_g"], f32) / np.sqrt(np.asarray(inputs["bn2_v"], f32) + BN_EPS)
    w1f = c1w * s1[:, :, None, None, None]
    w2f = c2w * s2[:, :, None, None, None]
    b1f = (np.asarray(inputs["conv1_b"], f32) - np.asarray(inputs["bn1_m"], f32)) * s1 \
        + np.asarray(inputs["bn1_b"], f32)
    b2f = (np.asarray(inputs["conv2_b"], f32) - np.asarray(inputs["bn2_m"], f32)) * s2 \
        + np.asarray(inputs["bn2_b"], f32)

    # pack stationary operands: [E, co, ci, ky, kx] -> [E, 128p, ci_t, 9, co]
    def pack_w(wf):
        return np.ascontiguousarray(
            wf.transpose(0, 2, 3, 4, 1)       # [E, ci, ky, kx, co]
            .reshape(E, CT, 128, 9, 256)
            .transpose(0, 2, 1, 3, 4))        # [E, 128, CT, 9, co]

    w1p = pack_w(w1f)
    w2p = pack_w(w2f)

    # padded input, cast to device dtype: [B, C, HP, WP]
    xp = np.zeros((B, C, HP, WP), np_sdt)
    xp[:, :, 1:1 + H, 1:1 + W] = moe_c4.astype(np_sdt)

    def split_mo(wp):
        # [E, 128, CT, 9, 256] -> [E, MT, 128, CT, 9, 128]
        return np.ascontiguousarray(
            wp.reshape(E, 128, CT, 9, MT, 128).transpose(0, 4, 1, 2, 3, 5))

    w_sel1 = split_mo(w1p)[top1].astype(np_sdt)  # [B, MT, 128, CT, 9, 128]
    w_sel2 = split_mo(w2p)[top1].astype(np_sdt)
    b_sel1 = b1f[top1].reshape(B, MT, 128)  # [B, MT, 128]
    b_sel2 = b2f[top1].reshape(B, MT, 128)

    in_maps = []
    for c in range(NCORES):
        sl = slice(c * SPC, (c + 1) * SPC)
        x_core = np.ascontiguousarray(
            xp[sl].reshape(SPC, CT, 128, HP, WP).transpose(0, 2, 1, 3, 4))
        w_core = np.ascontiguousarray(
            np.stack([w_sel1[sl], w_sel2[sl]], axis=1))  # [SPC, 2, MT, 128, CT, 9, 128]
        bias_core = np.ascontiguousarray(
            np.stack([b_sel1[sl], b_sel2[sl]], axis=1)   # [SPC, 2, MT, 128]
            .transpose(3, 0, 1, 2).reshape(128, SPC * 2 * MT)).astype(f32)
        in_maps.append({"x": x_core, "w": w_core, "bias": bias_core})

    nc = _get_nc(dt_mode)
    res = None
    for attempt in range(3):
        try:
            res = run_bass_kernel_spmd(
                nc, in_maps, core_ids=list(range(NCORES)),
                trace=trace,
                trace_cores=(trace_cores or [0]) if trace else None)
            break
        except Exception:
            if attempt == 2:
                raise
            time.sleep(5.0)

    out = np.concatenate(
        [r["out"].reshape(SPC, C, H, W) for r in res.results], axis=0)
    return (out.astype(f32), balance), res


def kernel(**inputs):
    (out, balance), _ = _run(inputs)
    return out, balance
